# revision 1
# baseline (speedup 1.0000x reference)
"""Trainium2 Bass kernel for virtual-node GAT attention (gnn_message_passing).

Reference semantics (N=100000, C=64, D=512, F=256):
    gh  = graph_node @ W            # (N, F)
    vh  = virtual_node @ W          # (C, F)
    e   = gh @ a1 + (vh @ a2)^T     # (N, C)
    e   = leaky_relu(e, 0.2)
    att = softmax(e, axis=1)
    out = att @ vh                  # (N, F)

Key algebraic identity: gh only enters via gh @ a1 = graph_node @ (W @ a1),
so the (N,D)@(D,F) matmul is never needed. Host precomputes the tiny shared
tables w1 = W@a1 (D,), vh (C,F), t = vh@a2 (C,); the device does the per-row
work: s = x·w1, e = lrelu(s + t), softmax over C, att @ vh. This makes the
kernel HBM-bound: each core streams 12.5k rows * (2KB in + 1KB out).

Device layout: each iteration covers 256 rows, striped so partition p owns
rows (2p, 2p+1) -> 4KB-contiguous input packets and 2KB output packets per
partition. DMAs are batched 7 iterations per instruction: each DMA's
completion semaphore (16 per-engine 4B writes with a WAW dependency on the
HBM store) stalls every SDMA engine ~1us, so fewer/bigger DMAs pack the
engines much tighter. Input DMAs ride the SP HWDGE ring; output DMAs ride
the ACT ring so the store stream's sequencer waits never stall the loads.

Per-iteration engine split (each engine stays under the ~2.9us/iter DMA):
  SP     in-DMA issue (1 per 7 iters)
  DVE    s = x.w1 (fused scalar_tensor_tensor x2), z = rowsum(exp),
         r = 1/z, att^T PSUM->SBUF copy
  ACT    prelu with fused +s bias (x2), exp (full width),
         h' PSUM->SBUF copies with fused 1/z scale (x2),
         out-DMA issue (1 per 7 iters)
  PE     att^T transpose (one per iter), h' matmuls (x2)

Sharding: graph_node rows split evenly across the 8 cores (data parallel),
small tables replicated. No cross-device communication.
"""

import numpy as np

N, D, F, C = 100000, 512, 256, 64
NCORES = 8
SHARD = N // NCORES            # 12500 rows per core
P = 128                        # partitions
RPI = 2 * P                    # rows per iteration (striped pairs)
ITERS = (SHARD + RPI - 1) // RPI   # 49
PAD = ITERS * RPI              # 12544 (pad shard with zero rows)
GRP = 8                        # max iterations per DMA batch
# Variable batch sizes: small first group -> compute starts sooner (short
# pipeline fill); tiny last group -> short drain tail after the final load.
# Even sizes so iterations batch into pairs (one Exp/rowsum/recip/att^T-copy
# per 512 rows); the final single iteration runs unpaired.
GROUPS = [4, 6, 8, 8, 8, 8, 6, 1]
assert sum(GROUPS) == ITERS
ALPHA = 0.2

_CACHE = {}


def _build_nc():
    import concourse.bacc as bacc
    import concourse.mybir as mybir
    import concourse.tile as tile

    fp32 = mybir.dt.float32
    Alu = mybir.AluOpType
    Act = mybir.ActivationFunctionType

    nc = bacc.Bacc("TRN2", target_bir_lowering=False, debug=False,
                   num_devices=NCORES)
    x = nc.dram_tensor("x", [PAD, D], fp32, kind="ExternalInput").ap()
    w1rep = nc.dram_tensor("w1rep", [P, D], fp32, kind="ExternalInput").ap()
    trep2 = nc.dram_tensor("trep2", [P, 2, C], fp32, kind="ExternalInput").ap()
    vh = nc.dram_tensor("vh", [C, F], fp32, kind="ExternalInput").ap()
    ident = nc.dram_tensor("ident", [P, P], fp32, kind="ExternalInput").ap()
    out = nc.dram_tensor("out", [PAD, F], fp32, kind="ExternalOutput").ap()

    with tile.TileContext(nc) as tc:
        with (
            tc.tile_pool(name="const", bufs=1) as constp,
            tc.tile_pool(name="xin", bufs=3) as xp,
            tc.tile_pool(name="prod", bufs=3) as prodp,
            tc.tile_pool(name="svec", bufs=8) as sp,
            tc.tile_pool(name="evec", bufs=6) as ep,
            tc.tile_pool(name="zvec", bufs=8) as zp,
            tc.tile_pool(name="pexp", bufs=4) as pexpp,
            tc.tile_pool(name="attT", bufs=4) as attp,
            tc.tile_pool(name="osb", bufs=3) as op_,
            tc.tile_pool(name="psT", bufs=2, space="PSUM") as psT,
            tc.tile_pool(name="psH", bufs=4, space="PSUM") as psH,
        ):
            w1_sb = constp.tile([P, D], fp32)
            nc.sync.dma_start(out=w1_sb, in_=w1rep)
            t2_sb = constp.tile([P, 2, C], fp32)
            nc.sync.dma_start(out=t2_sb, in_=trep2)
            # vh replicated in both partition halves: matmul requires lhsT
            # and rhs to share a base partition, and the att^T halves live
            # at partitions 0 and 64.
            vh_sb = constp.tile([P, F], fp32)
            nc.sync.dma_start(out=vh_sb[:C, :], in_=vh)
            nc.sync.dma_start(out=vh_sb[C:, :], in_=vh)
            id_sb = constp.tile([P, P], fp32)
            nc.sync.dma_start(out=id_sb, in_=ident)

            row0 = 0
            npair = 0
            for g, gsz in enumerate(GROUPS):
                xg = x[row0 * 2 * P:(row0 + gsz) * 2 * P, :].rearrange(
                    "(i p two) d -> p i two d", p=P, two=2)
                og = out[row0 * 2 * P:(row0 + gsz) * 2 * P, :].rearrange(
                    "(i p two) f -> p i two f", p=P, two=2)
                row0 += gsz
                xt = xp.tile([P, gsz, 2, D], fp32, tag="xt")
                nc.sync.dma_start(out=xt, in_=xg)
                osb = op_.tile([P, gsz, 2, F], fp32, tag="osb")
                i = 0
                while i < gsz:
                    nsub = min(2, gsz - i)   # iterations in this batch
                    nh = 2 * nsub            # 128-row halves in this batch
                    e4 = ep.tile([P, 4, C], fp32, tag="e4")
                    for k in range(nh):
                        prod = prodp.tile([P, D], fp32)
                        s = sp.tile([P, 1], fp32)
                        # s = sum_d x[:, d] * w1[d]  (prod is scratch; mul
                        # and row-reduce fuse into one DVE pass)
                        nc.vector.scalar_tensor_tensor(
                            out=prod, in0=xt[:, i + k // 2, k % 2, :],
                            scalar=1.0, in1=w1_sb, op0=Alu.mult,
                            op1=Alu.mult, accum_out=s)
                        # e = leaky_relu(t_j + s_i): Prelu honors alpha on
                        # HW (Lrelu's LUT bakes a fixed 0.01 slope) and
                        # fuses the per-partition bias add
                        nc.scalar.activation(
                            out=e4[:, k, :], in_=t2_sb[:, k % 2, :],
                            func=Act.Prelu, bias=s, scale=1.0, alpha=ALPHA)
                    pexp4 = pexpp.tile([P, 4, C], fp32, tag="pexp4")
                    nc.scalar.activation(out=pexp4[:, :nh, :],
                                         in_=e4[:, :nh, :], func=Act.Exp)
                    z4 = zp.tile([P, 4], fp32)
                    nc.vector.reduce_sum(z4[:, :nh], pexp4[:, :nh, :],
                                         axis=mybir.AxisListType.X)
                    r4 = zp.tile([P, 4], fp32, tag="r4")
                    nc.vector.reciprocal(r4[:, :nh], z4[:, :nh])
                    # One PE transpose per iteration (two halves at once):
                    # column h*64+j of pexp4[:, 2b:2b+2, :] becomes
                    # partition h*64+j of attT block b.
                    # [P, 2, 512]: each transpose output starts a PSUM bank
                    attT_ps = psT.tile([P, 2, 512], fp32)
                    for b in range(nsub):
                        nc.tensor.transpose(
                            attT_ps[:, b, :P],
                            pexp4.rearrange("p four c -> p (four c)")
                                 [:, 2 * b * C:(2 * b + 2) * C],
                            id_sb)
                    attT = attp.tile([P, 2, P], fp32)
                    nc.vector.tensor_copy(attT[:, :nsub, :],
                                          attT_ps[:, :nsub, :P])
                    for k in range(nh):
                        b, h = k // 2, k % 2
                        # h'_unnorm[p, :] for row 2p+h (matmul outputs must
                        # be bank-aligned -> one PSUM tile per half)
                        hp = psH.tile([P, F], fp32)
                        nc.tensor.matmul(
                            hp, attT[h * C:(h + 1) * C, b, :],
                            vh_sb[h * C:(h + 1) * C, :],
                            start=True, stop=True)
                        # normalize rows by 1/z during the PSUM->SBUF copy;
                        # every 5th pair sends one copy to DVE to even out
                        # the ACT/DVE load
                        if k == 3 and npair % 5 == 0:
                            nc.vector.tensor_scalar_mul(
                                osb[:, i + b, h, :], hp, r4[:, k:k + 1])
                        else:
                            nc.scalar.mul(osb[:, i + b, h, :], hp,
                                          r4[:, k:k + 1])
                    npair += 1
                    i += nsub
                    if gsz >= 6 and i == (gsz // 2 + 1) // 2 * 2:
                        # stagger: store the first half of the group as soon
                        # as its copies land, so the SDMA engines keep
                        # streaming during the group's compute tail
                        nc.scalar.dma_start(out=og[:, :i], in_=osb[:, :i])
                if gsz >= 6:
                    half = (gsz // 2 + 1) // 2 * 2
                    nc.scalar.dma_start(out=og[:, half:], in_=osb[:, half:])
                else:
                    # store via the ACT HWDGE ring (2KB/partition packets)
                    nc.scalar.dma_start(out=og, in_=osb)

    nc.compile()
    return nc


def _get_nc():
    if "nc" not in _CACHE:
        _CACHE["nc"] = _build_nc()
    return _CACHE["nc"]


def _prep_inputs(graph_node, virtual_node, W, a):
    f32 = np.float32
    W = np.asarray(W, f32)
    a = np.asarray(a, f32)
    a1 = a[:F, 0]
    a2 = a[F:, 0]
    w1 = (W @ a1).astype(f32)                       # (D,)
    vh = (np.asarray(virtual_node, f32) @ W).astype(f32)  # (C, F)
    t = (vh @ a2).astype(f32)                       # (C,)
    w1rep = np.ascontiguousarray(np.broadcast_to(w1, (P, D)), dtype=f32)
    trep2 = np.ascontiguousarray(
        np.broadcast_to(t, (P, 2, C)), dtype=f32)
    ident = np.eye(P, dtype=f32)

    X = np.asarray(graph_node, f32)
    in_maps = []
    for c in range(NCORES):
        xpad = np.zeros((PAD, D), f32)
        xpad[:SHARD] = X[c * SHARD:(c + 1) * SHARD]
        in_maps.append({"x": xpad, "w1rep": w1rep, "trep2": trep2,
                        "vh": np.ascontiguousarray(vh), "ident": ident})
    return in_maps


def _run(inputs, trace=False, **trace_kwargs):
    from concourse.bass_utils import run_bass_kernel_spmd

    nc = _get_nc()
    in_maps = _prep_inputs(**inputs)
    res = run_bass_kernel_spmd(nc, in_maps, list(range(NCORES)),
                               trace=trace, **trace_kwargs)
    out = np.concatenate(
        [res.results[c]["out"][:SHARD] for c in range(NCORES)], axis=0)
    return out, res


def kernel(**inputs) -> np.ndarray:
    out, _ = _run(inputs)
    return out



# revision 4
# speedup vs baseline: 1.6168x; 1.6168x over previous
"""Trainium2 Bass kernel for virtual-node GAT attention (gnn_message_passing).

Reference semantics (N=100000, C=64, D=512, F=256):
    gh  = graph_node @ W            # (N, F)
    vh  = virtual_node @ W          # (C, F)
    e   = gh @ a1 + (vh @ a2)^T     # (N, C)
    e   = leaky_relu(e, 0.2)
    att = softmax(e, axis=1)
    out = att @ vh                  # (N, F)

Algebraic identity: gh only enters via gh @ a1 = graph_node @ (W @ a1), so
the (N,D)@(D,F) matmul never happens. Host precomputes the tiny shared
tables w1 = W@a1 (D,), vh (C,F), t = vh@a2 (C,). The kernel is HBM-bound:
streaming x in and h' out once. Both streams ride bf16 (host casts), which
halves HBM traffic vs fp32; rel-err budget (2e-2) dwarfs bf16 noise.

Device pipeline, per 512-row block (x shipped TRANSPOSED by the host as
[4 d-chunks, 128, rows] bf16):
  PE   e^T[j, r] = sum_d w1[d] x[r, d]: 4 accumulating matmuls with
       lhsT = (w1 chunk) replicated across 64 columns, rhs = xT chunk.
       Output lands already transposed for the att matmul (no PE transpose,
       no DVE dot product). Two blocks pack one PSUM bank (partitions
       0-63 / 64-127).
  ACT  e = prelu(e^T + t) (bias=t fused, alpha honored), then exp -> bf16.
  PE   h'[r, :] = att^T.T @ [vh | 1]: the ones column makes col 256 the
       softmax denominator z (no reduction pass).
  DVE  r = 1/z; normalization fused into the PSUM->SBUF copies (split
       between ACT and DVE), writing bf16.

Host column permutation: xT column rc*128+rp holds row 4*rp+rc of its
block, so each h'-matmul output partition owns 4 consecutive HBM rows ->
2KB contiguous store packets and natural row order (no un-permute).

Sharding: rows split evenly across 8 cores (data parallel); small tables
replicated; no cross-device communication.
"""

import numpy as np

N, D, F, C = 100000, 512, 256, 64
NCORES = 8
SHARD = N // NCORES            # 12500 rows per core
P = 128
BLK = 512                      # rows per block (4 psum chunks of 128)
NBLK = 25                      # ceil(12500 / 512)
PADROWS = NBLK * BLK           # 12800
GROUPS_IN = [2, 4, 7, 7, 5]    # blocks per input DMA instruction
GROUPS_OUT = [4, 8, 8, 5]      # blocks per output DMA instruction
assert sum(GROUPS_IN) == NBLK and sum(GROUPS_OUT) == NBLK
ALPHA = 0.2

_CACHE = {}


def _build_nc():
    import concourse.bacc as bacc
    import concourse.mybir as mybir
    import concourse.tile as tile

    fp32 = mybir.dt.float32
    bf16 = mybir.dt.bfloat16
    Act = mybir.ActivationFunctionType

    nc = bacc.Bacc("TRN2", target_bir_lowering=False, debug=False,
                   num_devices=NCORES)
    xT = nc.dram_tensor("xT", [4, P, PADROWS], bf16, kind="ExternalInput").ap()
    wrep = nc.dram_tensor("wrep", [P, 4, C], bf16, kind="ExternalInput").ap()
    tcol = nc.dram_tensor("tcol", [P, 1], fp32, kind="ExternalInput").ap()
    vha = nc.dram_tensor("vha", [P, F + 1], bf16, kind="ExternalInput").ap()
    out = nc.dram_tensor("out", [PADROWS, F], bf16, kind="ExternalOutput").ap()

    # block -> (input group idx, local block idx); same for output groups
    gin_of, gout_of = {}, {}
    b = 0
    for g, gs in enumerate(GROUPS_IN):
        for i in range(gs):
            gin_of[b] = (g, i)
            b += 1
    b = 0
    for g, gs in enumerate(GROUPS_OUT):
        for i in range(gs):
            gout_of[b] = (g, i)
            b += 1
    gin_row0 = np.cumsum([0] + GROUPS_IN)
    gout_row0 = np.cumsum([0] + GROUPS_OUT)

    with tile.TileContext(nc) as tc:
        with (
            tc.tile_pool(name="const", bufs=1) as constp,
            tc.tile_pool(name="xin", bufs=2) as xp,
            tc.tile_pool(name="esb", bufs=2) as ep,
            tc.tile_pool(name="pexp", bufs=3) as pexpp,
            tc.tile_pool(name="rvec", bufs=4) as rp_,
            tc.tile_pool(name="osb", bufs=2) as op_,
            tc.tile_pool(name="psE", bufs=2, space="PSUM") as psE,
            tc.tile_pool(name="psH", bufs=3, space="PSUM") as psH,
        ):
            wrep_sb = constp.tile([P, 4, C], bf16)
            nc.sync.dma_start(out=wrep_sb, in_=wrep)
            tcol_sb = constp.tile([P, 1], fp32)
            nc.sync.dma_start(out=tcol_sb, in_=tcol)
            vha_sb = constp.tile([P, F + 1], bf16)
            nc.sync.dma_start(out=vha_sb, in_=vha)

            xt_tiles = [None] * len(GROUPS_IN)
            osb_tiles = [None] * len(GROUPS_OUT)

            def ensure_xt(b):
                g, _ = gin_of[b]
                if xt_tiles[g] is None:
                    gs = GROUPS_IN[g]
                    t = xp.tile([P, 4, gs * BLK], bf16, tag="xt", name="xt")
                    src = xT[:, :, gin_row0[g] * BLK:(gin_row0[g] + gs) * BLK]
                    nc.sync.dma_start(out=t, in_=src.rearrange("c p r -> p c r"))
                    xt_tiles[g] = t
                return xt_tiles[g], gin_of[b][1]

            def ensure_osb(b):
                g, _ = gout_of[b]
                if osb_tiles[g] is None:
                    gs = GROUPS_OUT[g]
                    osb_tiles[g] = op_.tile([P, gs, 4, F], bf16, tag="osb",
                                            name="osb")
                return osb_tiles[g], gout_of[b][1]

            for b0 in range(0, NBLK, 2):
                nh = min(2, NBLK - b0)       # blocks in this psum pair
                npart = nh * C               # active psum partitions
                pse = psE.tile([P, BLK], fp32)
                for h in range(nh):
                    xt, lb = ensure_xt(b0 + h)
                    for dc in range(4):
                        nc.tensor.matmul(
                            pse[h * C:(h + 1) * C, :],
                            wrep_sb[:, dc, :],
                            xt[:, dc, lb * BLK:(lb + 1) * BLK],
                            start=(dc == 0), stop=(dc == 3))
                esb = ep.tile([P, BLK], fp32, tag="esb")
                nc.scalar.activation(
                    out=esb[:npart, :], in_=pse[:npart, :], func=Act.Prelu,
                    bias=tcol_sb[:npart, :], scale=1.0, alpha=ALPHA)
                pex = pexpp.tile([P, BLK], bf16, tag="pex")
                nc.scalar.activation(out=pex[:npart, :], in_=esb[:npart, :],
                                     func=Act.Exp)
                for h in range(nh):
                    b = b0 + h
                    osb, ob = ensure_osb(b)
                    for cc in range(2):
                        ph = psH.tile([P, 2, BLK], fp32)
                        for i in range(2):
                            rc = cc * 2 + i
                            nc.tensor.matmul(
                                ph[:, i, :F + 1],
                                pex[h * C:(h + 1) * C, rc * P:(rc + 1) * P],
                                vha_sb[h * C:(h + 1) * C, :],
                                start=True, stop=True)
                        r2 = rp_.tile([P, 2], fp32)
                        nc.vector.reciprocal(r2, ph[:, :, F])
                        # normalize during PSUM->SBUF copy; split ACT/DVE
                        nc.scalar.mul(osb[:, ob, cc * 2, :],
                                      ph[:, 0, :F], r2[:, 0:1])
                        nc.vector.tensor_scalar_mul(osb[:, ob, cc * 2 + 1, :],
                                                    ph[:, 1, :F], r2[:, 1:2])
                    g, ob2 = gout_of[b]
                    if ob2 == GROUPS_OUT[g] - 1:
                        gs = GROUPS_OUT[g]
                        dst = out[gout_row0[g] * BLK:(gout_row0[g] + gs) * BLK, :]
                        nc.scalar.dma_start(
                            out=dst.rearrange("(b p four) f -> p b four f",
                                              four=4, p=P),
                            in_=osb_tiles[g])

    nc.compile()
    return nc


def _get_nc():
    if "nc" not in _CACHE:
        _CACHE["nc"] = _build_nc()
    return _CACHE["nc"]


def _prep_inputs(graph_node, virtual_node, W, a):
    import ml_dtypes
    f32 = np.float32
    bf16 = ml_dtypes.bfloat16
    W = np.asarray(W, f32)
    a = np.asarray(a, f32)
    a1 = a[:F, 0]
    a2 = a[F:, 0]
    w1 = (W @ a1).astype(f32)                             # (D,)
    vh = (np.asarray(virtual_node, f32) @ W).astype(f32)  # (C, F)
    t = (vh @ a2).astype(f32)                             # (C,)

    # wrep[p, dc, j] = w1[dc*128 + p] for all j (broadcast across columns)
    wrep = np.ascontiguousarray(
        np.broadcast_to(w1.reshape(4, P).T[:, :, None], (P, 4, C))
    ).astype(bf16)
    tcol = np.ascontiguousarray(np.concatenate([t, t])[:, None], dtype=f32)
    vha = np.ones((P, F + 1), f32)
    vha[:C, :F] = vh
    vha[C:, :F] = vh
    vha = vha.astype(bf16)

    X = np.asarray(graph_node, f32).astype(bf16)
    in_maps = []
    for core in range(NCORES):
        xpad = np.zeros((PADROWS, D), bf16)
        xpad[:SHARD] = X[core * SHARD:(core + 1) * SHARD]
        # xT[dc, dp, b*512 + rc*128 + rp] = x[b*512 + 4*rp + rc, dc*128 + dp]
        v = xpad.reshape(NBLK, P, 4, 4, P)       # [b, rp, rc, dc, dp]
        xT = np.ascontiguousarray(v.transpose(3, 4, 0, 2, 1)).reshape(
            4, P, PADROWS)
        in_maps.append({"xT": xT, "wrep": wrep, "tcol": tcol, "vha": vha})
    return in_maps


def _gather(results):
    return np.concatenate(
        [results[c]["out"][:SHARD].astype(np.float32) for c in range(NCORES)],
        axis=0)


def _run(inputs, trace=False, **trace_kwargs):
    from concourse.bass_utils import run_bass_kernel_spmd

    nc = _get_nc()
    in_maps = _prep_inputs(**inputs)
    res = run_bass_kernel_spmd(nc, in_maps, list(range(NCORES)),
                               trace=trace, **trace_kwargs)
    return _gather(res.results), res


def kernel(**inputs) -> np.ndarray:
    out, _ = _run(inputs)
    return out


# revision 12
# speedup vs baseline: 1.6743x; 1.0356x over previous
"""Trainium2 Bass kernel for virtual-node GAT attention (gnn_message_passing).

Reference semantics (N=100000, C=64, D=512, F=256):
    gh  = graph_node @ W            # (N, F)
    vh  = virtual_node @ W          # (C, F)
    e   = gh @ a1 + (vh @ a2)^T     # (N, C)
    e   = leaky_relu(e, 0.2)
    att = softmax(e, axis=1)
    out = att @ vh                  # (N, F)

Algebraic identity: gh only enters via gh @ a1 = graph_node @ (W @ a1), so
the (N,D)@(D,F) matmul never happens. Host precomputes the tiny shared
tables w1 = W@a1 (D,), vh (C,F), t = vh@a2 (C,). The kernel is HBM-bound:
streaming x in and h' out once. Both streams ride bf16 (host casts), which
halves HBM traffic vs fp32; rel-err budget (2e-2) dwarfs bf16 noise.

Device pipeline, per 512-row block (x shipped TRANSPOSED by the host as
[4 d-chunks, 128, rows] bf16):
  PE   e^T[j, r] = sum_d w1[d] x[r, d]: 4 accumulating matmuls with
       lhsT = (w1 chunk) replicated across 64 columns, rhs = xT chunk.
       Output lands already transposed for the att matmul (no PE transpose,
       no DVE dot product). Two blocks pack one PSUM bank (partitions
       0-63 / 64-127).
  ACT  e = prelu(e^T + t) (bias=t fused, alpha honored), then exp -> bf16.
  PE   h'[r, :] = att^T.T @ [vh | 1]: the ones column makes col 256 the
       softmax denominator z (no reduction pass).
  DVE  r = 1/z; normalization fused into the PSUM->SBUF copies (split
       between ACT and DVE), writing bf16.

Host column permutation: xT column rc*128+rp holds row 4*rp+rc of its
block, so each h'-matmul output partition owns 4 consecutive HBM rows ->
2KB contiguous store packets and natural row order (no un-permute).

Sharding: rows split evenly across 8 cores (data parallel); small tables
replicated; no cross-device communication.
"""

import numpy as np

N, D, F, C = 100000, 512, 256, 64
NCORES = 8
SHARD = N // NCORES            # 12500 rows per core
P = 128
BLK = 512                      # rows per block (4 psum chunks of 128)
NBLK = 25                      # ceil(12500 / 512)
PADROWS = NBLK * BLK           # 12800
# Pairs of blocks share one PSUM logits bank: (0), (1,2), ..., (23,24).
# The leading singleton lets compute start after a 1-block first DMA group.
GROUPS_IN = [1, 2, 4, 6, 6, 6]   # blocks per input DMA instruction
GROUPS_OUT = [3, 6, 6, 6, 2, 2]  # blocks per output DMA (small tail groups)
assert sum(GROUPS_IN) == NBLK and sum(GROUPS_OUT) == NBLK
ALPHA = 0.2

_CACHE = {}


def _build_nc():
    import concourse.bacc as bacc
    import concourse.mybir as mybir
    import concourse.tile as tile

    fp32 = mybir.dt.float32
    bf16 = mybir.dt.bfloat16
    fp16 = mybir.dt.float16
    Act = mybir.ActivationFunctionType

    nc = bacc.Bacc("TRN2", target_bir_lowering=False, debug=False,
                   num_devices=NCORES)
    # x and w1 ride fp16 (same bytes as bf16, 8x finer mantissa -> the
    # logits see ~8x less quantization noise). pexp/vha stay bf16: exp can
    # reach ~5e8 which overflows fp16.
    xT = nc.dram_tensor("xT", [4, P, PADROWS], fp16, kind="ExternalInput").ap()
    wrep = nc.dram_tensor("wrep", [P, 4, C], fp16, kind="ExternalInput").ap()
    tcol = nc.dram_tensor("tcol", [P, 1], fp32, kind="ExternalInput").ap()
    vha = nc.dram_tensor("vha", [P, F + 1], bf16, kind="ExternalInput").ap()
    out = nc.dram_tensor("out", [PADROWS, F], bf16, kind="ExternalOutput").ap()

    # block -> (input group idx, local block idx); same for output groups
    gin_of, gout_of = {}, {}
    b = 0
    for g, gs in enumerate(GROUPS_IN):
        for i in range(gs):
            gin_of[b] = (g, i)
            b += 1
    b = 0
    for g, gs in enumerate(GROUPS_OUT):
        for i in range(gs):
            gout_of[b] = (g, i)
            b += 1
    gin_row0 = np.cumsum([0] + GROUPS_IN)
    gout_row0 = np.cumsum([0] + GROUPS_OUT)

    with tile.TileContext(nc) as tc:
        with (
            tc.tile_pool(name="const", bufs=1) as constp,
            tc.tile_pool(name="xin", bufs=3) as xp,
            tc.tile_pool(name="esb", bufs=2) as ep,
            tc.tile_pool(name="pexp", bufs=3) as pexpp,
            tc.tile_pool(name="rvec", bufs=4) as rp_,
            tc.tile_pool(name="osb", bufs=3) as op_,
            tc.tile_pool(name="psE", bufs=2, space="PSUM") as psE,
            tc.tile_pool(name="psH", bufs=3, space="PSUM") as psH,
        ):
            # consts ride the ACT HWDGE ring: the x stream owns the SP ring
            # from its first instruction, and the ACT ring is idle at start
            wrep_sb = constp.tile([P, 4, C], fp16)
            nc.scalar.dma_start(out=wrep_sb, in_=wrep)
            tcol_sb = constp.tile([P, 1], fp32)
            nc.scalar.dma_start(out=tcol_sb, in_=tcol)
            vha_sb = constp.tile([P, F + 1], bf16)
            nc.scalar.dma_start(out=vha_sb, in_=vha)

            xt_tiles = [None] * len(GROUPS_IN)
            osb_tiles = [None] * len(GROUPS_OUT)

            def ensure_xt(b):
                g, _ = gin_of[b]
                if xt_tiles[g] is None:
                    gs = GROUPS_IN[g]
                    t = xp.tile([P, 4, gs * BLK], fp16, tag="xt", name="xt")
                    src = xT[:, :, gin_row0[g] * BLK:(gin_row0[g] + gs) * BLK]
                    nc.sync.dma_start(out=t, in_=src.rearrange("c p r -> p c r"))
                    xt_tiles[g] = t
                return xt_tiles[g], gin_of[b][1]

            def ensure_osb(b):
                g, _ = gout_of[b]
                if osb_tiles[g] is None:
                    gs = GROUPS_OUT[g]
                    osb_tiles[g] = op_.tile([P, gs, 4, F], bf16, tag="osb",
                                            name="osb")
                return osb_tiles[g], gout_of[b][1]

            pairs = [(0,)] + [(b, b + 1) for b in range(1, NBLK, 2)]
            for pair in pairs:
                nh = len(pair)               # blocks in this psum pair
                npart = nh * C               # active psum partitions
                pse = psE.tile([P, BLK], fp32)
                for h in range(nh):
                    xt, lb = ensure_xt(pair[h])
                    for dc in range(4):
                        nc.tensor.matmul(
                            pse[h * C:(h + 1) * C, :],
                            wrep_sb[:, dc, :],
                            xt[:, dc, lb * BLK:(lb + 1) * BLK],
                            start=(dc == 0), stop=(dc == 3))
                esb = ep.tile([P, BLK], fp32, tag="esb")
                nc.scalar.activation(
                    out=esb[:npart, :], in_=pse[:npart, :], func=Act.Prelu,
                    bias=tcol_sb[:npart, :], scale=1.0, alpha=ALPHA)
                pex = pexpp.tile([P, BLK], bf16, tag="pex")
                nc.scalar.activation(out=pex[:npart, :], in_=esb[:npart, :],
                                     func=Act.Exp)
                for h in range(nh):
                    b = pair[h]
                    osb, ob = ensure_osb(b)
                    for cc in range(2):
                        ph = psH.tile([P, 2, BLK], fp32)
                        for i in range(2):
                            rc = cc * 2 + i
                            nc.tensor.matmul(
                                ph[:, i, :F + 1],
                                pex[h * C:(h + 1) * C, rc * P:(rc + 1) * P],
                                vha_sb[h * C:(h + 1) * C, :],
                                start=True, stop=True)
                        r2 = rp_.tile([P, 2], fp32)
                        nc.vector.reciprocal(r2, ph[:, :, F])
                        # normalize during PSUM->SBUF copy; split ACT/DVE
                        # (GPSIMD cannot read PSUM)
                        nc.scalar.mul(osb[:, ob, cc * 2, :],
                                      ph[:, 0, :F], r2[:, 0:1])
                        nc.vector.tensor_scalar_mul(osb[:, ob, cc * 2 + 1, :],
                                                    ph[:, 1, :F], r2[:, 1:2])
                    g, ob2 = gout_of[b]
                    if ob2 == GROUPS_OUT[g] - 1:
                        gs = GROUPS_OUT[g]
                        dst = out[gout_row0[g] * BLK:(gout_row0[g] + gs) * BLK, :]
                        nc.scalar.dma_start(
                            out=dst.rearrange("(b p four) f -> p b four f",
                                              four=4, p=P),
                            in_=osb_tiles[g])

    nc.compile()
    return nc


def _get_nc():
    if "nc" not in _CACHE:
        _CACHE["nc"] = _build_nc()
    return _CACHE["nc"]


def _prep_inputs(graph_node, virtual_node, W, a):
    import ml_dtypes
    f32 = np.float32
    bf16 = ml_dtypes.bfloat16
    W = np.asarray(W, f32)
    a = np.asarray(a, f32)
    a1 = a[:F, 0]
    a2 = a[F:, 0]
    w1 = (W @ a1).astype(f32)                             # (D,)
    vh = (np.asarray(virtual_node, f32) @ W).astype(f32)  # (C, F)
    t = (vh @ a2).astype(f32)                             # (C,)

    # wrep[p, dc, j] = w1[dc*128 + p] for all j (broadcast across columns)
    wrep = np.ascontiguousarray(
        np.broadcast_to(w1.reshape(4, P).T[:, :, None], (P, 4, C))
    ).astype(np.float16)
    tcol = np.ascontiguousarray(np.concatenate([t, t])[:, None], dtype=f32)
    vha = np.ones((P, F + 1), f32)
    vha[:C, :F] = vh
    vha[C:, :F] = vh
    vha = vha.astype(bf16)

    X = np.asarray(graph_node, f32).astype(np.float16)
    in_maps = []
    for core in range(NCORES):
        xpad = np.zeros((PADROWS, D), np.float16)
        xpad[:SHARD] = X[core * SHARD:(core + 1) * SHARD]
        # xT[dc, dp, b*512 + rc*128 + rp] = x[b*512 + 4*rp + rc, dc*128 + dp]
        v = xpad.reshape(NBLK, P, 4, 4, P)       # [b, rp, rc, dc, dp]
        xT = np.ascontiguousarray(v.transpose(3, 4, 0, 2, 1)).reshape(
            4, P, PADROWS)
        in_maps.append({"xT": xT, "wrep": wrep, "tcol": tcol, "vha": vha})
    return in_maps


def _gather(results):
    return np.concatenate(
        [results[c]["out"][:SHARD].astype(np.float32) for c in range(NCORES)],
        axis=0)


def _run(inputs, trace=False, **trace_kwargs):
    from concourse.bass_utils import run_bass_kernel_spmd

    nc = _get_nc()
    in_maps = _prep_inputs(**inputs)
    res = run_bass_kernel_spmd(nc, in_maps, list(range(NCORES)),
                               trace=trace, **trace_kwargs)
    return _gather(res.results), res


def kernel(**inputs) -> np.ndarray:
    out, _ = _run(inputs)
    return out
